# revision 7
# baseline (speedup 1.0000x reference)
"""Trainium2 Bass kernel for the BDH recurrent block (B=8, T=256, d=256, n=1024).

Closed-form reformulation (one sample per NeuronCore, data-parallel over B):

  U = relu(emb @ Dx.T)                      (T, n)
  x_t = sum_s C[t,s] U_s                    C[t,s] = 0.97^{t-s} / prod_{r=s..t} b_r
  G = X X^T,  GD = G o DupT                 (decay-masked gram)
  a*_t = (GD @ ln(emb))_t                   rows are exactly zero-mean
  y = relu(Dy a*^T) o X^T   -- the 1/sqrt(var+eps) factor of ln(a*) is a
      positive per-t scalar that commutes through relu/mul/matmul-over-n; it
      is computed in a side chain and re-applied to the v rows just before
      the final layernorm (exact, verified against the reference in fp64).
  v = ln_rs(y^T E^T)

All heavy matmuls run in bf16; the log-space cumsum chain (b, logb,
Lambda, q) stays exact f32.  Masks / triangles / decay tables are generated
on device from iota.

DMA: the queues are packet-rate limited (~290ns per 16-engine round up to
~4KB rows), so inputs are packed into two mega-tensors with ~5KB rows --
one per HWDGE queue -- and the output is row-split across both queues.

C^T[s,t] = exp(q_s + grid_t + mask[s,t]) with
  q_s    = Lambda_{s-1}                (exclusive cumsum of log b, f32 PE)
  grid_t = -Lambda_t                   (inclusive, broadcast along s via a
                                        replicated -logb stationary matmul)
  mask   = (t-s) ln 0.97 + [s>t] -inf
The t>=128 half of grid is emitted relative to total0 = sum(logb[0:128]);
total0 moves into the per-partition exp bias of that half.
"""

import numpy as np
import ml_dtypes

import concourse.bass as bass
import concourse.tile as tile
from concourse import bacc, mybir
from concourse.bass_utils import run_bass_kernel_spmd
from concourse.hw_specs import get_activation_tables

B, T, D, N = 8, 256, 256, 1024
P = 128
LN_EPS = 1e-5
DECAY = 0.97
LN097 = float(np.log(np.float64(DECAY)))
BIG = 1e28
F32 = mybir.dt.float32
F32R = mybir.dt.float32r
BF = mybir.dt.bfloat16
AF = mybir.ActivationFunctionType
ALU = mybir.AluOpType

N_WARM = 32   # dummy PE matmuls to lift the HAM clock gate during DMA wait
UXW = 8 + D + N        # per-half width of the ux pack (sc | embT | dxT)
WDW = D + N            # per-half width of the wd pack (emb | dyT)


def _build_nc():
    nc = bacc.Bacc(enable_partition_id=False)

    # ux row p = [sc_p | emb.T_p | Dx.T_p | (same for p+128)]
    d_ux = nc.dram_tensor("ux", [P, 2 * UXW], BF, kind="ExternalInput")
    # wd row p = [emb_p | Dy.T_p | (same for p+128)]
    d_wd = nc.dram_tensor("wd", [P, 2 * WDW], BF, kind="ExternalInput")
    # eT row p = [E.T[k*128+p, :] for k in 0..7]
    d_eT = nc.dram_tensor("eT", [P, 8 * D], BF, kind="ExternalInput")
    # out row p = [out[p, :] | out[p+128, :]]
    d_out = nc.dram_tensor("out", [P, 2 * D], F32, kind="ExternalOutput")

    act_sets = list(get_activation_tables(nc.m.arch))
    combined_set_id = act_sets.index("natural_log_exp_and_others")

    with tile.TileContext(nc) as tc:
        nc.scalar.add_instruction(mybir.InstLoadActFuncSet(
            name=nc.get_next_instruction_name(),
            act_func_set_id=combined_set_id, ins=[], outs=[]))
        with (
            tc.tile_pool(name="consts", bufs=1) as cp,
            tc.tile_pool(name="work", bufs=1) as wp,
            tc.tile_pool(name="ps512", bufs=3, space="PSUM") as ps512,
            tc.tile_pool(name="psA", bufs=3, space="PSUM") as psA,
            tc.tile_pool(name="pss", bufs=1, space="PSUM") as pss,
        ):
            # ---- DMA issues (per-engine program order == queue order) ----
            ux = cp.tile([P, 2 * UXW], BF, tag="ux", name="ux")
            nc.sync.dma_start(ux[:], d_ux[:, :])
            wd = cp.tile([P, 2 * WDW], BF, tag="wd", name="wd")
            nc.scalar.dma_start(wd[:], d_wd[:, :])
            eTt = cp.tile([P, 8 * D], BF, tag="eTt", name="eTt")
            nc.sync.dma_start(eTt[:], d_eT[:, :])
            ET_s = [eTt[:, m * D:(m + 1) * D] for m in range(8)]

            c097_s = [ux[:, k * UXW:k * UXW + 2].bitcast(F32)
                      for k in range(2)]
            embT_s = [ux[:, k * UXW + 8:k * UXW + 8 + D] for k in range(2)]
            dxT_c = {(k, ch): ux[:, k * UXW + 8 + D + ch * 512:
                                 k * UXW + 8 + D + (ch + 1) * 512]
                     for k in range(2) for ch in range(2)}
            emb_s = [wd[:, k * WDW:k * WDW + D] for k in range(2)]
            dyT_s = [wd[:, k * WDW + D:(k + 1) * WDW] for k in range(2)]

            # ---- on-device constants ------------------------------------
            ones_bfb = cp.tile([P, P], BF, tag="ones_bfb", name="ones_bfb")
            nc.vector.memset(ones_bfb[:], 1.0)
            ones_bfc = cp.tile([P, 1], BF, tag="ones_bfc", name="ones_bfc")
            nc.vector.memset(ones_bfc[:], 1.0)
            ones_blk = cp.tile([P, P], F32, tag="ones_blk", name="ones_blk")
            nc.vector.memset(ones_blk[:], 1.0)
            zero_col = cp.tile([P, 1], F32, tag="zero_col", name="zero_col")
            nc.vector.memset(zero_col[:], 0.0)
            eps_col = cp.tile([P, 1], F32, tag="eps_col", name="eps_col")
            nc.vector.memset(eps_col[:], LN_EPS)
            negln_col = cp.tile([P, 1], F32, tag="negln_col", name="negln_col")
            nc.vector.memset(negln_col[:], -LN097)

            # shared small-psum bank: cumsums, var sums
            pst = pss.tile([P, 8], F32, tag="pss", name="pst")
            pl0, tcol, pl1 = pst[:, 0:1], pst[:, 1:2], pst[:, 2:3]
            var_tiles = [pst[:, 3:4], pst[:, 4:5]]

            # PE warm-up: 128-col bf16 matmuls (~107ns cold each) keep the
            # PE busy through the DMA wait so the HAM clock gate opens
            # (K=8/8, 2.4 GHz) before the real matmuls begin.  Results are
            # dead writes into the grid bank (its real use comes later).
            gridt = psA.tile([P, T], F32, tag="grid", name="gridt", bufs=1)
            for i in range(N_WARM):
                nc.tensor.matmul(gridt[:, 0:P], ones_bfb[:], ones_bfb[:],
                                 start=True, stop=True)

            # iota grids: iox[c][p, j] = j - p - 128*c   (f32, exact)
            iox = []
            for c in range(2):
                t = cp.tile([P, 2 * P], F32, tag=f"iox{c}", name=f"iox{c}")
                nc.gpsimd.iota(t[:], pattern=[[1, 2 * P]], base=-P * c,
                               channel_multiplier=-1,
                               allow_small_or_imprecise_dtypes=True)
                iox.append(t)

            # triangles (DVE):  triu[r,s] = [r < s]  (exclusive, q-cumsum)
            #                   triuI[r,s] = [r <= s] (inclusive, grid)
            triu = cp.tile([P, P], F32, tag="triu", name="triu")
            nc.vector.tensor_scalar(triu[:], iox[0][:, 0:P], 1.0, None,
                                    op0=ALU.is_ge)
            triuI = cp.tile([P, P], F32, tag="triuI", name="triuI")
            nc.vector.tensor_scalar(triuI[:], iox[0][:, 0:P], 0.0, None,
                                    op0=ALU.is_ge)

            # maskCT[c][s,t] = (t-s)*ln097 + min(t-s,0)*BIG    (s = 128c + p)
            maskCT = []
            for c in range(2):
                u = wp.tile([P, 2 * P], F32, tag=f"msku{c}", name=f"msku{c}")
                nc.vector.tensor_scalar(u[:], iox[c][:], 0.0, BIG,
                                        op0=ALU.min, op1=ALU.mult)
                m = cp.tile([P, 2 * P], F32, tag=f"mask{c}", name=f"mask{c}")
                nc.vector.scalar_tensor_tensor(
                    out=m[:], in0=iox[c][:], scalar=LN097, in1=u[:],
                    op0=ALU.mult, op1=ALU.add)
                maskCT.append(m)

            # DupT[c][s,t] = 0.97^(t-1-s) for s<t else 0
            #   = exp(maskCT - ln097 + min(t-s-0.5, 0)*BIG)
            # (DVE args here; ACT exps emitted after the CT exps)
            dup = []
            dupw_s = []
            for c in range(2):
                u2 = wp.tile([P, 2 * P], F32, tag=f"dupu{c}", name=f"dupu{c}")
                nc.vector.tensor_scalar(u2[:], iox[c][:], 0.5, 0.0,
                                        op0=ALU.subtract, op1=ALU.min)
                w2 = wp.tile([P, 2 * P], F32, tag=f"dupw{c}", name=f"dupw{c}")
                nc.vector.scalar_tensor_tensor(
                    out=w2[:], in0=u2[:], scalar=BIG, in1=maskCT[c][:],
                    op0=ALU.mult, op1=ALU.add)
                dupw_s.append(w2)
                dup.append(
                    cp.tile([P, 2 * P], F32, tag=f"dup{c}", name=f"dup{c}"))

            # ---- U = relu(emb @ Dx.T), row sums ------------------------
            U_s = [wp.tile([P, N], F32R, tag=f"U{m}", name=f"U{m}")
                   for m in range(2)]
            pu_tiles = {}
            apA = [[None] * 2 for _ in range(2)]
            apB = [[None] * 2 for _ in range(2)]
            for mt in range(2):
                for ch in range(2):
                    pu_tiles[(mt, ch)] = ps512.tile(
                        [P, 512], F32, tag="pu", name="pu")
                    for k in range(2):
                        nc.tensor.matmul(
                            pu_tiles[(mt, ch)][:],
                            embT_s[k][:, mt * P:(mt + 1) * P],
                            dxT_c[(k, ch)][:],
                            start=(k == 0), stop=(k == 1))
                # relu + row-sum accum, column-split across ACT and DVE
                for ch in range(2):
                    a = wp.tile([P, 1], F32, tag=f"apA{mt}{ch}",
                                name=f"apA{mt}{ch}")
                    nc.scalar.activation(
                        out=U_s[mt][:, ch * 512:ch * 512 + 256],
                        in_=pu_tiles[(mt, ch)][:, 0:256],
                        func=AF.Relu, bias=zero_col[:], accum_out=a[:])
                    apA[mt][ch] = a
                    b_ = wp.tile([P, 1], F32, tag=f"apB{mt}{ch}",
                                 name=f"apB{mt}{ch}")
                    nc.vector.tensor_scalar(
                        U_s[mt][:, ch * 512 + 256:(ch + 1) * 512],
                        pu_tiles[(mt, ch)][:, 256:512], 0.0, 0.0,
                        op0=ALU.max, op1=ALU.add, accum_out=b_[:])
                    apB[mt][ch] = b_

            # ---- b, logb; replicated -logb tiles ------------------------
            logb_s, rep_s = [], []
            for mt in range(2):
                s1 = wp.tile([P, 1], F32, tag=f"s1{mt}", name=f"s1{mt}")
                nc.vector.scalar_tensor_tensor(
                    out=s1[:], in0=apA[mt][0][:], scalar=c097_s[mt][:],
                    in1=apB[mt][0][:], op0=ALU.add, op1=ALU.add)
                bvec = wp.tile([P, 1], F32, tag=f"b{mt}", name=f"b{mt}")
                nc.vector.scalar_tensor_tensor(
                    out=bvec[:], in0=apA[mt][1][:], scalar=s1[:],
                    in1=apB[mt][1][:], op0=ALU.add, op1=ALU.add)
                lb = wp.tile([P, 1], F32, tag=f"lb{mt}", name=f"lb{mt}")
                nc.scalar.activation(out=lb[:], in_=bvec[:], func=AF.Ln,
                                     bias=zero_col[:])
                logb_s.append(lb)
                rep = wp.tile([P, P], F32, tag=f"rep{mt}", name=f"rep{mt}")
                nc.vector.tensor_scalar(rep[:], ones_blk[:], lb[:], -1.0,
                                        op0=ALU.mult, op1=ALU.mult)
                rep_s.append(rep)

            # ---- cumsums on PE (exact f32) ------------------------------
            glo = gridt[:, 0:P]
            ghi = gridt[:, P:2 * P]
            nc.tensor.matmul(pl0, triu[:], logb_s[0][:],
                             start=True, stop=True)
            nc.tensor.matmul(glo, rep_s[0][:], triuI[:],
                             start=True, stop=True)
            nc.tensor.matmul(tcol, ones_blk[:], logb_s[0][:],
                             start=True, stop=True)
            nc.tensor.matmul(pl1, triu[:], logb_s[1][:],
                             start=True, stop=True)
            nc.tensor.matmul(ghi, rep_s[1][:], triuI[:],
                             start=True, stop=True)

            # biases: lo half q_st = Lambda_{s-1};  hi half q2 = q - total0
            q0 = wp.tile([P, 1], F32, tag="q0", name="q0")
            nc.vector.tensor_scalar(q0[:], pl0, 0.0, None, op0=ALU.add)
            q2_0 = wp.tile([P, 1], F32, tag="q2_0", name="q2_0")
            nc.vector.tensor_sub(q2_0[:], q0[:], tcol)
            q2_1 = wp.tile([P, 1], F32, tag="q2_1", name="q2_1")
            nc.vector.tensor_scalar(q2_1[:], pl1, 0.0, None, op0=ALU.add)
            q1 = wp.tile([P, 1], F32, tag="q1", name="q1")
            nc.vector.tensor_add(q1[:], q2_1[:], tcol)
            bias_lo = [q0, q1]
            bias_hi = [q2_0, q2_1]

            # ---- CT[s,t] = exp(grid + mask + bias) ----------------------
            CT_s = []
            tmp_s = []
            for st in range(2):
                tmp = wp.tile([P, T], F32, tag=f"ctmp{st}", name=f"ctmp{st}")
                nc.vector.tensor_add(tmp[:, 0:P], glo, maskCT[st][:, 0:P])
                nc.vector.tensor_add(tmp[:, P:2 * P], ghi,
                                     maskCT[st][:, P:2 * P])
                tmp_s.append(tmp)
                CT_s.append(
                    wp.tile([P, T], F32R, tag=f"CT{st}", name=f"CT{st}"))
            for st in range(2):
                nc.scalar.activation(out=CT_s[st][:, 0:P],
                                     in_=tmp_s[st][:, 0:P], func=AF.Exp,
                                     bias=bias_lo[st][:])
                nc.scalar.activation(out=CT_s[st][:, P:2 * P],
                                     in_=tmp_s[st][:, P:2 * P], func=AF.Exp,
                                     bias=bias_hi[st][:])
            for c in range(2):
                nc.scalar.activation(out=dup[c][:], in_=dupw_s[c][:],
                                     func=AF.Exp, bias=negln_col[:])

            # ---- W = ln(emb rows)  (DVE stats + ACT rsqrt chain) -------
            # scheduled no earlier than ~12us sim-time so its DVE/ACT ops
            # don't get committed into the critical b->logb->CT chain
            W_s = []
            w_ctx = tc.tile_wait_until(0.012)
            w_ctx.__enter__()
            for mt in range(2):
                st6w = wp.tile([P, 6], F32, tag=f"st6w{mt}", name=f"st6w{mt}")
                nc.vector.bn_stats(st6w[:], emb_s[mt][:])
                mvw = wp.tile([P, 2], F32, tag=f"mvw{mt}", name=f"mvw{mt}")
                nc.vector.bn_aggr(mvw[:], st6w[:])
                lvw = wp.tile([P, 1], F32, tag=f"lvw{mt}", name=f"lvw{mt}")
                nc.scalar.activation(out=lvw[:], in_=mvw[:, 1:2], func=AF.Ln,
                                     bias=eps_col[:])
                rsw = wp.tile([P, 1], F32, tag=f"rsw{mt}", name=f"rsw{mt}")
                nc.scalar.activation(out=rsw[:], in_=lvw[:], func=AF.Exp,
                                     bias=zero_col[:], scale=-0.5)
                nmr = wp.tile([P, 1], F32, tag=f"nmr{mt}", name=f"nmr{mt}")
                nc.vector.tensor_scalar(nmr[:], mvw[:, 0:1], rsw[:], -1.0,
                                        op0=ALU.mult, op1=ALU.mult)
                w = wp.tile([P, D], F32R, tag=f"W{mt}", name=f"W{mt}")
                nc.scalar.activation(out=w[:], in_=emb_s[mt][:],
                                     func=AF.Identity, bias=nmr[:],
                                     scale=rsw[:])
                W_s.append(w)
            w_ctx.__exit__(None, None, None)

            # ---- X^T = U^T C^T  (n on partitions, T free) --------------
            XT_s = []
            for m in range(8):
                px = psA.tile([P, T], F32, tag="ps", name="px")
                for k in range(2):
                    nc.tensor.matmul(px[:], U_s[k][:, m * P:(m + 1) * P],
                                     CT_s[k][:], start=(k == 0), stop=(k == 1))
                xt = wp.tile([P, T], F32R, tag=f"XT{m}", name=f"XT{m}")
                if m % 2 == 0:
                    nc.vector.tensor_copy(xt[:], px[:])
                else:
                    nc.scalar.copy(xt[:], px[:])
                XT_s.append(xt)

            # ---- G = X X^T ; GD = G o DupT ------------------------------
            GD_s = []
            for st in range(2):
                pg = psA.tile([P, T], F32, tag="ps", name="pg")
                for k in range(8):
                    nc.tensor.matmul(pg[:], XT_s[k][:, st * P:(st + 1) * P],
                                     XT_s[k][:], start=(k == 0), stop=(k == 7))
                gd = wp.tile([P, T], F32R, tag=f"GD{st}", name=f"GD{st}")
                nc.vector.tensor_mul(gd[:], pg[:], dup[st][:])
                GD_s.append(gd)

            # ---- A^T = W^T @ GD  ([d, t], layernorm-free) ---------------
            AT_s = []
            sq_s = []
            for k in range(2):
                pa = psA.tile([P, T], F32, tag="ps", name="pa")
                for sc in range(2):
                    nc.tensor.matmul(pa[:], W_s[sc][:, k * P:(k + 1) * P],
                                     GD_s[sc][:], start=(sc == 0),
                                     stop=(sc == 1))
                at = wp.tile([P, T], BF, tag=f"AT{k}", name=f"AT{k}")
                nc.vector.tensor_copy(at[:], pa[:])
                AT_s.append(at)
                sq = wp.tile([P, T], BF, tag=f"sqA{k}", name=f"sqA{k}")
                nc.scalar.activation(out=sq[:], in_=pa[:], func=AF.Square,
                                     bias=zero_col[:])
                sq_s.append(sq)

            # ---- deferred LN scale: rs_t = rsqrt(mean_d(A^2)+eps) -------
            for mt in range(2):
                for k in range(2):
                    nc.tensor.matmul(var_tiles[mt],
                                     sq_s[k][:, mt * P:(mt + 1) * P],
                                     ones_bfc[:], start=(k == 0),
                                     stop=(k == 1))

            # ---- y^T = relu(Dy A^T) o X^T  (relu on ACT, mult on DVE) ---
            yT_s = []
            for m in range(8):
                py = psA.tile([P, T], F32, tag="ps", name="py")
                for k in range(2):
                    nc.tensor.matmul(py[:], dyT_s[k][:, m * P:(m + 1) * P],
                                     AT_s[k][:], start=(k == 0), stop=(k == 1))
                ry = wp.tile([P, T], F32R, tag=f"ry{m}", name=f"ry{m}")
                nc.scalar.activation(out=ry[:], in_=py[:], func=AF.Relu,
                                     bias=zero_col[:])
                yt = wp.tile([P, T], BF, tag=f"yT{m}", name=f"yT{m}")
                nc.vector.tensor_mul(yt[:], ry[:].bitcast(F32),
                                     XT_s[m][:].bitcast(F32))
                yT_s.append(yt)

            rs_s = []
            for mt in range(2):
                lvv = wp.tile([P, 1], F32, tag=f"lvv{mt}", name=f"lvv{mt}")
                nc.scalar.activation(out=lvv[:], in_=var_tiles[mt],
                                     func=AF.Ln, bias=eps_col[:],
                                     scale=1.0 / D)
                rs = wp.tile([P, 1], F32, tag=f"rs{mt}", name=f"rs{mt}")
                nc.scalar.activation(out=rs[:], in_=lvv[:], func=AF.Exp,
                                     bias=zero_col[:], scale=-0.5)
                rs_s.append(rs)

            # ---- v = y E^T ; final LN with deferred rs ------------------
            pv_s, mv_s, uv_s = [], [], []
            for mt in range(2):
                pv = psA.tile([P, D], F32, tag="ps", name="pv")
                for m in range(8):
                    nc.tensor.matmul(pv[:], yT_s[m][:, mt * P:(mt + 1) * P],
                                     ET_s[m][:], start=(m == 0), stop=(m == 7))
                pv_s.append(pv)
            for mt in range(2):
                st6 = wp.tile([P, 6], F32, tag=f"ost{mt}", name=f"ost{mt}")
                nc.vector.bn_stats(st6[:], pv_s[mt][:])
                mv = wp.tile([P, 2], F32, tag=f"omv{mt}", name=f"omv{mt}")
                nc.vector.bn_aggr(mv[:], st6[:])
                mv_s.append(mv)
                rs2 = wp.tile([P, 1], F32, tag=f"rs2{mt}", name=f"rs2{mt}")
                nc.vector.tensor_scalar(rs2[:], rs_s[mt][:], rs_s[mt][:],
                                        None, op0=ALU.mult)
                uv = wp.tile([P, 1], F32, tag=f"uv{mt}", name=f"uv{mt}")
                nc.vector.scalar_tensor_tensor(
                    out=uv[:], in0=rs2[:], scalar=mv[:, 1:2], in1=eps_col[:],
                    op0=ALU.mult, op1=ALU.add)
                uv_s.append(uv)
            sv_s = []
            for mt in range(2):
                lv = wp.tile([P, 1], F32, tag=f"olv{mt}", name=f"olv{mt}")
                nc.scalar.activation(out=lv[:], in_=uv_s[mt][:], func=AF.Ln,
                                     bias=zero_col[:])
                sv = wp.tile([P, 1], F32, tag=f"osv{mt}", name=f"osv{mt}")
                nc.scalar.activation(out=sv[:], in_=lv[:], func=AF.Exp,
                                     bias=zero_col[:], scale=-0.5)
                sv_s.append(sv)
            ovb = wp.tile([P, 2 * D], F32, tag="ovb", name="ovb")
            sf_s = []
            for mt in range(2):
                sfin = wp.tile([P, 1], F32, tag=f"sf{mt}", name=f"sf{mt}")
                nc.vector.tensor_scalar(sfin[:], sv_s[mt][:], rs_s[mt][:],
                                        None, op0=ALU.mult)
                sf_s.append(sfin)
            nms0 = wp.tile([P, 1], F32, tag="nms0", name="nms0")
            nc.vector.tensor_scalar(nms0[:], mv_s[0][:, 0:1], sf_s[0][:],
                                    -1.0, op0=ALU.mult, op1=ALU.mult)
            nc.scalar.activation(out=ovb[:, 0:D], in_=pv_s[0][:],
                                 func=AF.Identity, bias=nms0[:],
                                 scale=sf_s[0][:])
            nc.vector.tensor_scalar(ovb[:, D:2 * D], pv_s[1][:],
                                    mv_s[1][:, 0:1], sf_s[1][:],
                                    op0=ALU.subtract, op1=ALU.mult)
            # output row-split across both queues
            nc.sync.dma_start(d_out[0:P // 2, :], ovb[0:P // 2, :])
            nc.scalar.dma_start(d_out[P // 2:P, :], ovb[P // 2:P, :])

    nc.finalize()
    return nc


_NC_CACHE = {}


def _get_nc():
    if "nc" not in _NC_CACHE:
        _NC_CACHE["nc"] = _build_nc()
    return _NC_CACHE["nc"]


def make_in_maps(embeddings, E, Dx, Dy):
    bf = ml_dtypes.bfloat16
    emb = np.ascontiguousarray(np.asarray(embeddings, dtype=np.float32))
    E = np.asarray(E, dtype=np.float32)
    Dx = np.asarray(Dx, dtype=np.float32)
    Dy = np.asarray(Dy, dtype=np.float32)

    sc = np.zeros((T, 4), np.float32)
    sc[:, 0] = DECAY
    sc[0, 0] = 0.0
    sc_bf = sc.view(bf)  # bit-reinterpret: [T, 8] bf16

    def fold(a):  # [256, W] -> [128, 2W]: row p = [row_p | row_{p+128}]
        return np.ascontiguousarray(np.concatenate([a[:P], a[P:]], axis=1))

    ET = np.ascontiguousarray(E.T)                       # [N, D]
    eT = np.ascontiguousarray(
        ET.reshape(8, P, D).transpose(1, 0, 2).reshape(P, 8 * D).astype(bf))

    DxT = Dx.T.astype(bf)
    DyT = Dy.T.astype(bf)
    in_maps = []
    for b in range(B):
        embb = emb[b].astype(bf)
        ux = fold(np.concatenate([sc_bf, emb[b].T.astype(bf), DxT], axis=1))
        wdp = fold(np.concatenate([embb, DyT], axis=1))
        in_maps.append({"ux": ux, "wd": wdp, "eT": eT})
    return in_maps


def unfold_out(o):  # [128, 2D] -> [256, D]
    return np.concatenate([o[:, 0:D], o[:, D:2 * D]], axis=0)


def kernel(embeddings, E, Dx, Dy):
    in_maps = make_in_maps(embeddings, E, Dx, Dy)
    nc = _get_nc()
    res = run_bass_kernel_spmd(nc, in_maps, core_ids=list(range(B)))
    return np.stack([unfold_out(r["out"]) for r in res.results], axis=0)


# revision 8
# speedup vs baseline: 1.0102x; 1.0102x over previous
"""Trainium2 Bass kernel for the BDH recurrent block (B=8, T=256, d=256, n=1024).

Closed-form reformulation (one sample per NeuronCore, data-parallel over B):

  U = relu(emb @ Dx.T)                      (T, n)
  x_t = sum_s C[t,s] U_s                    C[t,s] = 0.97^{t-s} / prod_{r=s..t} b_r
  G = X X^T,  GD = G o DupT                 (decay-masked gram)
  a*_t = (GD @ ln(emb))_t                   rows are exactly zero-mean
  y = relu(Dy a*^T) o X^T   -- the 1/sqrt(var+eps) factor of ln(a*) is a
      positive per-t scalar that commutes through relu/mul/matmul-over-n; it
      is computed in a side chain and re-applied to the v rows just before
      the final layernorm (exact, verified against the reference in fp64).
  v = ln_rs(y^T E^T)

All heavy matmuls run in bf16; the log-space cumsum chain (b, logb,
Lambda, q) stays exact f32.  Masks / triangles / decay tables are generated
on device from iota.

DMA: the queues are packet-rate limited (~290ns per 16-engine round up to
~4KB rows), so inputs are packed into two mega-tensors with ~5KB rows --
one per HWDGE queue -- and the output is row-split across both queues.

C^T[s,t] = exp(q_s + grid_t + mask[s,t]) with
  q_s    = Lambda_{s-1}                (exclusive cumsum of log b, f32 PE)
  grid_t = -Lambda_t                   (inclusive, broadcast along s via a
                                        replicated -logb stationary matmul)
  mask   = (t-s) ln 0.97 + [s>t] -inf
The t>=128 half of grid is emitted relative to total0 = sum(logb[0:128]);
total0 moves into the per-partition exp bias of that half.
"""

import numpy as np
import ml_dtypes

import concourse.bass as bass
import concourse.tile as tile
from concourse import bacc, mybir
from concourse.bass_utils import run_bass_kernel_spmd
from concourse.hw_specs import get_activation_tables

B, T, D, N = 8, 256, 256, 1024
P = 128
LN_EPS = 1e-5
DECAY = 0.97
LN097 = float(np.log(np.float64(DECAY)))
BIG = 1e28
F32 = mybir.dt.float32
F32R = mybir.dt.float32r
BF = mybir.dt.bfloat16
AF = mybir.ActivationFunctionType
ALU = mybir.AluOpType

N_WARM = 32   # dummy PE matmuls to lift the HAM clock gate during DMA wait
UXW = 8 + D + N        # per-half width of the ux pack (sc | embT | dxT)
WDW = D + N            # per-half width of the wd pack (emb | dyT)


def _build_nc():
    nc = bacc.Bacc(enable_partition_id=False)

    # ux row p = [sc_p | emb.T_p | Dx.T_p | (same for p+128)]
    d_ux = nc.dram_tensor("ux", [P, 2 * UXW], BF, kind="ExternalInput")
    # wd row p = [emb_p | Dy.T_p | (same for p+128)]
    d_wd = nc.dram_tensor("wd", [P, 2 * WDW], BF, kind="ExternalInput")
    # eT row p = [E.T[k*128+p, :] for k in 0..7]
    d_eT = nc.dram_tensor("eT", [P, 8 * D], BF, kind="ExternalInput")
    # out row p = [out[p, :] | out[p+128, :]]
    d_out = nc.dram_tensor("out", [P, 2 * D], F32, kind="ExternalOutput")

    act_sets = list(get_activation_tables(nc.m.arch))
    combined_set_id = act_sets.index("natural_log_exp_and_others")

    with tile.TileContext(nc) as tc:
        nc.scalar.add_instruction(mybir.InstLoadActFuncSet(
            name=nc.get_next_instruction_name(),
            act_func_set_id=combined_set_id, ins=[], outs=[]))
        with (
            tc.tile_pool(name="consts", bufs=1) as cp,
            tc.tile_pool(name="work", bufs=1) as wp,
            tc.tile_pool(name="ps512", bufs=3, space="PSUM") as ps512,
            tc.tile_pool(name="psA", bufs=2, space="PSUM") as psA,
            tc.tile_pool(name="psG", bufs=2, space="PSUM") as psG,
        ):
            # ---- DMA issues (per-engine program order == queue order) ----
            ux = cp.tile([P, 2 * UXW], BF, tag="ux", name="ux")
            nc.sync.dma_start(ux[:], d_ux[:, :])
            wd = cp.tile([P, 2 * WDW], BF, tag="wd", name="wd")
            nc.scalar.dma_start(wd[:], d_wd[:, :])
            eTt = cp.tile([P, 8 * D], BF, tag="eTt", name="eTt")
            nc.sync.dma_start(eTt[:], d_eT[:, :])
            ET_s = [eTt[:, m * D:(m + 1) * D] for m in range(8)]

            c097_s = [ux[:, k * UXW:k * UXW + 2].bitcast(F32)
                      for k in range(2)]
            embT_s = [ux[:, k * UXW + 8:k * UXW + 8 + D] for k in range(2)]
            dxT_c = {(k, ch): ux[:, k * UXW + 8 + D + ch * 512:
                                 k * UXW + 8 + D + (ch + 1) * 512]
                     for k in range(2) for ch in range(2)}
            emb_s = [wd[:, k * WDW:k * WDW + D] for k in range(2)]
            dyT_s = [wd[:, k * WDW + D:(k + 1) * WDW] for k in range(2)]

            # ---- on-device constants ------------------------------------
            ones_bfb = cp.tile([P, P], BF, tag="ones_bfb", name="ones_bfb")
            nc.vector.memset(ones_bfb[:], 1.0)
            ones_bfc = cp.tile([P, 1], BF, tag="ones_bfc", name="ones_bfc")
            nc.vector.memset(ones_bfc[:], 1.0)
            ones_blk = cp.tile([P, P], F32, tag="ones_blk", name="ones_blk")
            nc.vector.memset(ones_blk[:], 1.0)
            zero_col = cp.tile([P, 1], F32, tag="zero_col", name="zero_col")
            nc.vector.memset(zero_col[:], 0.0)
            eps_col = cp.tile([P, 1], F32, tag="eps_col", name="eps_col")
            nc.vector.memset(eps_col[:], LN_EPS)
            negln_col = cp.tile([P, 1], F32, tag="negln_col", name="negln_col")
            nc.vector.memset(negln_col[:], -LN097)

            # one psum bank holds the grid plus all [128,1] outputs
            gridt = psA.tile([P, T + 8], F32, tag="grid", name="gridt",
                             bufs=1)
            pl0, tcol, pl1 = (gridt[:, T:T + 1], gridt[:, T + 1:T + 2],
                              gridt[:, T + 2:T + 3])
            var_tiles = [gridt[:, T + 3:T + 4], gridt[:, T + 4:T + 5]]

            # PE warm-up: 128-col bf16 matmuls (~107ns cold each) keep the
            # PE busy through the DMA wait so the HAM clock gate opens
            # (K=8/8, 2.4 GHz) before the real matmuls begin.  Results are
            # dead writes into the grid bank (its real use comes later).
            for i in range(N_WARM):
                nc.tensor.matmul(gridt[:, 0:P], ones_bfb[:], ones_bfb[:],
                                 start=True, stop=True)

            # iota grids: iox[c][p, j] = j - p - 128*c   (f32, exact)
            iox = []
            for c in range(2):
                t = cp.tile([P, 2 * P], F32, tag=f"iox{c}", name=f"iox{c}")
                nc.gpsimd.iota(t[:], pattern=[[1, 2 * P]], base=-P * c,
                               channel_multiplier=-1,
                               allow_small_or_imprecise_dtypes=True)
                iox.append(t)

            # triangles (DVE):  triu[r,s] = [r < s]  (exclusive, q-cumsum)
            #                   triuI[r,s] = [r <= s] (inclusive, grid)
            triu = cp.tile([P, P], F32, tag="triu", name="triu")
            nc.vector.tensor_scalar(triu[:], iox[0][:, 0:P], 1.0, None,
                                    op0=ALU.is_ge)
            triuI = cp.tile([P, P], F32, tag="triuI", name="triuI")
            nc.vector.tensor_scalar(triuI[:], iox[0][:, 0:P], 0.0, None,
                                    op0=ALU.is_ge)

            # maskCT[c][s,t] = (t-s)*ln097 + min(t-s,0)*BIG    (s = 128c + p)
            maskCT = []
            for c in range(2):
                u = wp.tile([P, 2 * P], F32, tag=f"msku{c}", name=f"msku{c}")
                nc.vector.tensor_scalar(u[:], iox[c][:], 0.0, BIG,
                                        op0=ALU.min, op1=ALU.mult)
                m = cp.tile([P, 2 * P], F32, tag=f"mask{c}", name=f"mask{c}")
                nc.vector.scalar_tensor_tensor(
                    out=m[:], in0=iox[c][:], scalar=LN097, in1=u[:],
                    op0=ALU.mult, op1=ALU.add)
                maskCT.append(m)

            # DupT[c][s,t] = 0.97^(t-1-s) for s<t else 0
            #   = exp(maskCT - ln097 + min(t-s-0.5, 0)*BIG)
            # (DVE args here; ACT exps emitted after the CT exps)
            dup = []
            dupw_s = []
            for c in range(2):
                u2 = wp.tile([P, 2 * P], F32, tag=f"dupu{c}", name=f"dupu{c}")
                nc.vector.tensor_scalar(u2[:], iox[c][:], 0.5, 0.0,
                                        op0=ALU.subtract, op1=ALU.min)
                w2 = wp.tile([P, 2 * P], F32, tag=f"dupw{c}", name=f"dupw{c}")
                nc.vector.scalar_tensor_tensor(
                    out=w2[:], in0=u2[:], scalar=BIG, in1=maskCT[c][:],
                    op0=ALU.mult, op1=ALU.add)
                dupw_s.append(w2)
                dup.append(
                    cp.tile([P, 2 * P], F32, tag=f"dup{c}", name=f"dup{c}"))

            # ---- U = relu(emb @ Dx.T), row sums ------------------------
            U_s = [wp.tile([P, N], F32R, tag=f"U{m}", name=f"U{m}")
                   for m in range(2)]
            pu_tiles = {}
            apart = [[wp.tile([P, 1], F32, tag=f"ap{m}{c}", name=f"ap{m}{c}")
                      for c in range(2)] for m in range(2)]
            with tc.high_priority():
                for mt in range(2):
                    for ch in range(2):
                        pu_tiles[(mt, ch)] = ps512.tile(
                            [P, 512], F32, tag="pu", name="pu")
                        for k in range(2):
                            nc.tensor.matmul(
                                pu_tiles[(mt, ch)][:],
                                embT_s[k][:, mt * P:(mt + 1) * P],
                                dxT_c[(k, ch)][:],
                                start=(k == 0), stop=(k == 1))
                    # relu + row-sum accum; ch0 on ACT, ch1 on DVE
                    nc.scalar.activation(
                        out=U_s[mt][:, 0:512], in_=pu_tiles[(mt, 0)][:],
                        func=AF.Relu, bias=zero_col[:],
                        accum_out=apart[mt][0][:])
                    nc.vector.tensor_scalar(
                        U_s[mt][:, 512:1024], pu_tiles[(mt, 1)][:], 0.0, 0.0,
                        op0=ALU.max, op1=ALU.add, accum_out=apart[mt][1][:])

            # ---- b, logb; replicated -logb tiles ------------------------
            logb_s, rep_s = [], []
            hp_ctx = tc.high_priority()
            hp_ctx.__enter__()
            for mt in range(2):
                bvec = wp.tile([P, 1], F32, tag=f"b{mt}", name=f"b{mt}")
                nc.vector.scalar_tensor_tensor(
                    out=bvec[:], in0=apart[mt][0][:], scalar=c097_s[mt][:],
                    in1=apart[mt][1][:], op0=ALU.add, op1=ALU.add)
                lb = wp.tile([P, 1], F32, tag=f"lb{mt}", name=f"lb{mt}")
                nc.scalar.activation(out=lb[:], in_=bvec[:], func=AF.Ln,
                                     bias=zero_col[:])
                logb_s.append(lb)
                rep = wp.tile([P, P], F32, tag=f"rep{mt}", name=f"rep{mt}")
                nc.vector.tensor_scalar(rep[:], ones_blk[:], lb[:], -1.0,
                                        op0=ALU.mult, op1=ALU.mult)
                rep_s.append(rep)

            # ---- cumsums on PE (exact f32) ------------------------------
            glo = gridt[:, 0:P]
            ghi = gridt[:, P:2 * P]
            nc.tensor.matmul(pl0, triu[:], logb_s[0][:],
                             start=True, stop=True)
            nc.tensor.matmul(glo, rep_s[0][:], triuI[:],
                             start=True, stop=True)
            nc.tensor.matmul(tcol, ones_blk[:], logb_s[0][:],
                             start=True, stop=True)
            nc.tensor.matmul(pl1, triu[:], logb_s[1][:],
                             start=True, stop=True)
            nc.tensor.matmul(ghi, rep_s[1][:], triuI[:],
                             start=True, stop=True)

            # biases: lo half q_st = Lambda_{s-1};  hi half q2 = q - total0
            q0 = wp.tile([P, 1], F32, tag="q0", name="q0")
            nc.vector.tensor_scalar(q0[:], pl0, 0.0, None, op0=ALU.add)
            q2_0 = wp.tile([P, 1], F32, tag="q2_0", name="q2_0")
            nc.vector.tensor_sub(q2_0[:], q0[:], tcol)
            q2_1 = wp.tile([P, 1], F32, tag="q2_1", name="q2_1")
            nc.vector.tensor_scalar(q2_1[:], pl1, 0.0, None, op0=ALU.add)
            q1 = wp.tile([P, 1], F32, tag="q1", name="q1")
            nc.vector.tensor_add(q1[:], q2_1[:], tcol)
            bias_lo = [q0, q1]
            bias_hi = [q2_0, q2_1]

            # ---- CT[s,t] = exp(grid + mask + bias) ----------------------
            CT_s = []
            tmp_s = []
            for st in range(2):
                tmp = wp.tile([P, T], F32, tag=f"ctmp{st}", name=f"ctmp{st}")
                nc.vector.tensor_add(tmp[:, 0:P], glo, maskCT[st][:, 0:P])
                nc.vector.tensor_add(tmp[:, P:2 * P], ghi,
                                     maskCT[st][:, P:2 * P])
                tmp_s.append(tmp)
                CT_s.append(
                    wp.tile([P, T], F32R, tag=f"CT{st}", name=f"CT{st}"))
            for st in range(2):
                nc.scalar.activation(out=CT_s[st][:, 0:P],
                                     in_=tmp_s[st][:, 0:P], func=AF.Exp,
                                     bias=bias_lo[st][:])
                nc.scalar.activation(out=CT_s[st][:, P:2 * P],
                                     in_=tmp_s[st][:, P:2 * P], func=AF.Exp,
                                     bias=bias_hi[st][:])
            hp_ctx.__exit__(None, None, None)
            for c in range(2):
                nc.scalar.activation(out=dup[c][:], in_=dupw_s[c][:],
                                     func=AF.Exp, bias=negln_col[:])

            # ---- W = ln(emb rows)  (DVE stats + ACT rsqrt chain) -------
            W_s = []
            for mt in range(2):
                st6w = wp.tile([P, 6], F32, tag=f"st6w{mt}", name=f"st6w{mt}")
                nc.vector.bn_stats(st6w[:], emb_s[mt][:])
                mvw = wp.tile([P, 2], F32, tag=f"mvw{mt}", name=f"mvw{mt}")
                nc.vector.bn_aggr(mvw[:], st6w[:])
                lvw = wp.tile([P, 1], F32, tag=f"lvw{mt}", name=f"lvw{mt}")
                nc.scalar.activation(out=lvw[:], in_=mvw[:, 1:2], func=AF.Ln,
                                     bias=eps_col[:])
                rsw = wp.tile([P, 1], F32, tag=f"rsw{mt}", name=f"rsw{mt}")
                nc.scalar.activation(out=rsw[:], in_=lvw[:], func=AF.Exp,
                                     bias=zero_col[:], scale=-0.5)
                nmr = wp.tile([P, 1], F32, tag=f"nmr{mt}", name=f"nmr{mt}")
                nc.vector.tensor_scalar(nmr[:], mvw[:, 0:1], rsw[:], -1.0,
                                        op0=ALU.mult, op1=ALU.mult)
                w = wp.tile([P, D], F32R, tag=f"W{mt}", name=f"W{mt}")
                nc.scalar.activation(out=w[:], in_=emb_s[mt][:],
                                     func=AF.Identity, bias=nmr[:],
                                     scale=rsw[:])
                W_s.append(w)

            # ---- X^T = U^T C^T, interleaved with G = X X^T --------------
            # XT psum tiles cycle the psA pool; G accumulates in psG tiles
            # whose long 8-matmul groups hide the weight-load latency.
            XT_s = [None] * 8
            pg_s = [None] * 2

            def emit_xt(m):
                px = psA.tile([P, T], F32, tag="ps", name="px")
                for k in range(2):
                    nc.tensor.matmul(px[:], U_s[k][:, m * P:(m + 1) * P],
                                     CT_s[k][:], start=(k == 0), stop=(k == 1))
                xt = wp.tile([P, T], F32R, tag=f"XT{m}", name=f"XT{m}")
                if m % 2 == 0:
                    nc.vector.tensor_copy(xt[:], px[:])
                else:
                    nc.scalar.copy(xt[:], px[:])
                XT_s[m] = xt

            def emit_g(st, m):
                if m == 0:
                    pg_s[st] = psG.tile([P, T], F32, tag="psg", name="pg")
                nc.tensor.matmul(pg_s[st][:],
                                 XT_s[m][:, st * P:(st + 1) * P],
                                 XT_s[m][:], start=(m == 0), stop=(m == 7))

            emit_xt(0)
            emit_xt(1)
            emit_xt(2)
            emit_g(0, 0)
            emit_xt(3)
            emit_g(0, 1)
            emit_g(1, 0)
            emit_xt(4)
            emit_g(0, 2)
            emit_g(1, 1)
            emit_xt(5)
            emit_g(0, 3)
            emit_g(1, 2)
            emit_xt(6)
            emit_g(0, 4)
            emit_g(1, 3)
            emit_xt(7)
            emit_g(0, 5)
            emit_g(1, 4)
            emit_g(0, 6)
            emit_g(1, 5)
            emit_g(0, 7)
            emit_g(1, 6)
            emit_g(1, 7)

            GD_s = []
            for st in range(2):
                gd = wp.tile([P, T], F32R, tag=f"GD{st}", name=f"GD{st}")
                nc.vector.tensor_mul(gd[:], pg_s[st][:], dup[st][:])
                GD_s.append(gd)

            # ---- A^T = W^T @ GD  ([d, t], layernorm-free) ---------------
            AT_s = []
            sq_s = []
            for k in range(2):
                pa = psG.tile([P, T], F32, tag="psg", name="pa")
                for sc in range(2):
                    nc.tensor.matmul(pa[:], W_s[sc][:, k * P:(k + 1) * P],
                                     GD_s[sc][:], start=(sc == 0),
                                     stop=(sc == 1))
                at = wp.tile([P, T], BF, tag=f"AT{k}", name=f"AT{k}")
                nc.vector.tensor_copy(at[:], pa[:])
                AT_s.append(at)
                sq = wp.tile([P, T], BF, tag=f"sqA{k}", name=f"sqA{k}")
                nc.scalar.activation(out=sq[:], in_=pa[:], func=AF.Square,
                                     bias=zero_col[:])
                sq_s.append(sq)

            # ---- deferred LN scale: rs_t = rsqrt(mean_d(A^2)+eps) -------
            for mt in range(2):
                for k in range(2):
                    nc.tensor.matmul(var_tiles[mt],
                                     sq_s[k][:, mt * P:(mt + 1) * P],
                                     ones_bfc[:], start=(k == 0),
                                     stop=(k == 1))

            # ---- y^T = relu(Dy A^T) o X^T, interleaved with v = y E^T ---
            yT_s = [None] * 8
            pv_s = [None] * 2

            def emit_yt(m):
                py = psA.tile([P, T], F32, tag="ps", name="py")
                for k in range(2):
                    nc.tensor.matmul(py[:], dyT_s[k][:, m * P:(m + 1) * P],
                                     AT_s[k][:], start=(k == 0), stop=(k == 1))
                ry = wp.tile([P, T], F32R, tag=f"ry{m}", name=f"ry{m}")
                nc.scalar.activation(out=ry[:], in_=py[:], func=AF.Relu,
                                     bias=zero_col[:])
                yt = wp.tile([P, T], BF, tag=f"yT{m}", name=f"yT{m}")
                nc.vector.tensor_mul(yt[:], ry[:].bitcast(F32),
                                     XT_s[m][:].bitcast(F32))
                yT_s[m] = yt

            def emit_v(mt, m):
                if m == 0:
                    pv_s[mt] = psG.tile([P, D], F32, tag="psg", name="pv")
                nc.tensor.matmul(pv_s[mt][:],
                                 yT_s[m][:, mt * P:(mt + 1) * P],
                                 ET_s[m][:], start=(m == 0), stop=(m == 7))

            emit_yt(0)
            emit_yt(1)
            emit_yt(2)
            emit_v(0, 0)
            emit_yt(3)
            emit_v(0, 1)
            emit_v(1, 0)
            emit_yt(4)
            emit_v(0, 2)
            emit_v(1, 1)
            emit_yt(5)
            emit_v(0, 3)
            emit_v(1, 2)
            emit_yt(6)
            emit_v(0, 4)
            emit_v(1, 3)
            emit_yt(7)
            emit_v(0, 5)
            emit_v(1, 4)
            emit_v(0, 6)
            emit_v(1, 5)
            emit_v(0, 7)
            emit_v(1, 6)
            emit_v(1, 7)

            rs_s = []
            for mt in range(2):
                lvv = wp.tile([P, 1], F32, tag=f"lvv{mt}", name=f"lvv{mt}")
                nc.scalar.activation(out=lvv[:], in_=var_tiles[mt],
                                     func=AF.Ln, bias=eps_col[:],
                                     scale=1.0 / D)
                rs = wp.tile([P, 1], F32, tag=f"rs{mt}", name=f"rs{mt}")
                nc.scalar.activation(out=rs[:], in_=lvv[:], func=AF.Exp,
                                     bias=zero_col[:], scale=-0.5)
                rs_s.append(rs)

            # ---- final LN with deferred rs ------------------------------
            mv_s, uv_s = [], []
            for mt in range(2):
                st6 = wp.tile([P, 6], F32, tag=f"ost{mt}", name=f"ost{mt}")
                nc.vector.bn_stats(st6[:], pv_s[mt][:])
                mv = wp.tile([P, 2], F32, tag=f"omv{mt}", name=f"omv{mt}")
                nc.vector.bn_aggr(mv[:], st6[:])
                mv_s.append(mv)
                rs2 = wp.tile([P, 1], F32, tag=f"rs2{mt}", name=f"rs2{mt}")
                nc.vector.tensor_scalar(rs2[:], rs_s[mt][:], rs_s[mt][:],
                                        None, op0=ALU.mult)
                uv = wp.tile([P, 1], F32, tag=f"uv{mt}", name=f"uv{mt}")
                nc.vector.scalar_tensor_tensor(
                    out=uv[:], in0=rs2[:], scalar=mv[:, 1:2], in1=eps_col[:],
                    op0=ALU.mult, op1=ALU.add)
                uv_s.append(uv)
            sv_s = []
            for mt in range(2):
                lv = wp.tile([P, 1], F32, tag=f"olv{mt}", name=f"olv{mt}")
                nc.scalar.activation(out=lv[:], in_=uv_s[mt][:], func=AF.Ln,
                                     bias=zero_col[:])
                sv = wp.tile([P, 1], F32, tag=f"osv{mt}", name=f"osv{mt}")
                nc.scalar.activation(out=sv[:], in_=lv[:], func=AF.Exp,
                                     bias=zero_col[:], scale=-0.5)
                sv_s.append(sv)
            ovb = wp.tile([P, 2 * D], F32, tag="ovb", name="ovb")
            sf_s = []
            for mt in range(2):
                sfin = wp.tile([P, 1], F32, tag=f"sf{mt}", name=f"sf{mt}")
                nc.vector.tensor_scalar(sfin[:], sv_s[mt][:], rs_s[mt][:],
                                        None, op0=ALU.mult)
                sf_s.append(sfin)
            nms0 = wp.tile([P, 1], F32, tag="nms0", name="nms0")
            nc.vector.tensor_scalar(nms0[:], mv_s[0][:, 0:1], sf_s[0][:],
                                    -1.0, op0=ALU.mult, op1=ALU.mult)
            nc.scalar.activation(out=ovb[:, 0:D], in_=pv_s[0][:],
                                 func=AF.Identity, bias=nms0[:],
                                 scale=sf_s[0][:])
            nc.vector.tensor_scalar(ovb[:, D:2 * D], pv_s[1][:],
                                    mv_s[1][:, 0:1], sf_s[1][:],
                                    op0=ALU.subtract, op1=ALU.mult)
            # output row-split across both queues
            nc.sync.dma_start(d_out[0:P // 2, :], ovb[0:P // 2, :])
            nc.scalar.dma_start(d_out[P // 2:P, :], ovb[P // 2:P, :])

    nc.finalize()
    return nc


_NC_CACHE = {}


def _get_nc():
    if "nc" not in _NC_CACHE:
        _NC_CACHE["nc"] = _build_nc()
    return _NC_CACHE["nc"]


def make_in_maps(embeddings, E, Dx, Dy):
    bf = ml_dtypes.bfloat16
    emb = np.ascontiguousarray(np.asarray(embeddings, dtype=np.float32))
    E = np.asarray(E, dtype=np.float32)
    Dx = np.asarray(Dx, dtype=np.float32)
    Dy = np.asarray(Dy, dtype=np.float32)

    sc = np.zeros((T, 4), np.float32)
    sc[:, 0] = DECAY
    sc[0, 0] = 0.0
    sc_bf = sc.view(bf)  # bit-reinterpret: [T, 8] bf16

    def fold(a):  # [256, W] -> [128, 2W]: row p = [row_p | row_{p+128}]
        return np.ascontiguousarray(np.concatenate([a[:P], a[P:]], axis=1))

    ET = np.ascontiguousarray(E.T)                       # [N, D]
    eT = np.ascontiguousarray(
        ET.reshape(8, P, D).transpose(1, 0, 2).reshape(P, 8 * D).astype(bf))

    DxT = Dx.T.astype(bf)
    DyT = Dy.T.astype(bf)
    in_maps = []
    for b in range(B):
        embb = emb[b].astype(bf)
        ux = fold(np.concatenate([sc_bf, emb[b].T.astype(bf), DxT], axis=1))
        wdp = fold(np.concatenate([embb, DyT], axis=1))
        in_maps.append({"ux": ux, "wd": wdp, "eT": eT})
    return in_maps


def unfold_out(o):  # [128, 2D] -> [256, D]
    return np.concatenate([o[:, 0:D], o[:, D:2 * D]], axis=0)


def kernel(embeddings, E, Dx, Dy):
    in_maps = make_in_maps(embeddings, E, Dx, Dy)
    nc = _get_nc()
    res = run_bass_kernel_spmd(nc, in_maps, core_ids=list(range(B)))
    return np.stack([unfold_out(r["out"]) for r in res.results], axis=0)
